# revision 43
# baseline (speedup 1.0000x reference)
"""Trainium2 Bass kernel for AIMv2FlashAttention2 (packed varlen attention).

Problem: hidden [8192, 1024] = 8 packed sequences x 1024 tokens, dim=1024,
16 heads x 64 head_dim. qkv proj + RoPE (rotate-half) + block-diagonal
softmax attention + out proj.

Strategy: pure data parallelism -- attention is block-diagonal per sequence,
so each of the 8 NeuronCores processes one full sequence locally with
replicated weights. Zero collectives.

v10 structure (exp-floor-paced stream; fused per-step exp):
  - scores for a step live in ONE [128, 2, 1024] PSUM tile (4 banks,
    bufs=1): 8 K=32 QK matmuls on 4 PE row-groups, then a SINGLE
    ScalarE exp over all 2048 columns ((2048+352)/1.2 = 2.0us vs
    2x1.15 split) -- ScalarE stops being the step pacer.
  - per step: QK (2 rounds of 4 row-tiled matmuls) -> fused exp ->
    softmax-sum quad (4 col-tiled M=1 matmuls, concurrent) -> PV
    (2 rounds of 2 col-tiled M=64), PV/sums trail QK by LAG=2 steps.
  - feeder work (qkv chunks, v chunks, per-seg out-proj, norm
    broadcasts) is spliced between attention blocks by a weighted burn
    (2 sites/step); qk feeders yield at quarter-chunk granularity so
    no step absorbs more than ~1us of feeder work (the old full-chunk
    bursts made feeder steps 2x slower than the exp floor).
  - out-proj restructured per segment: as soon as segment (g, ih)
    normalizes, its outT chunk pair (2g, 2g+1) is projected for the 4
    token chunks of q-half ih and accumulated into fp32 y_acc; the
    g==3 contribution adds y_acc and streams the output out per
    (tc, eh) half on alternating HWDGE rings. Only segment (3,1)'s
    projection trails the stream (~7us tail).
  - normalization: sums staged to SBUF (one wide DVE copy, high
    priority), broadcast via one-hot K=128 matmul, reciprocal, single
    multiply evacuates PSUM->outT; drained AFTER the next segment's QK
    block so the Rs matmul never head-of-line blocks the QK/exp pacer.
  - PSUM: scores 1x[128,2048] (4 banks), pv 2x[128,512], sums 1 bank,
    feeder scratch 1 bank.
  - bf16 output (host casts back to fp32): halves the output DMA.
"""

import numpy as np
import ml_dtypes

import concourse.bass as bass
import concourse.bacc as bacc
import concourse.mybir as mybir
import concourse.tile as tile
from concourse.bass import ts

F32 = mybir.dt.float32
F16 = mybir.dt.bfloat16

P = 128
L = 1024          # tokens per sequence / core
DIM = 1024
H = 16            # heads
D = 64            # head dim
NCORES = 8
LAG = 2           # PV trails QK by this many jc steps


def build_nc(dbg=False):
    nc = bacc.Bacc(None)

    xT = nc.declare_dram_parameter("xT", [DIM, L], F16, isOutput=False)
    wqk = nc.declare_dram_parameter("wqk", [16, P, DIM], F16, isOutput=False)
    wv = nc.declare_dram_parameter("wv", [8, P, DIM], F16, isOutput=False)
    wp = nc.declare_dram_parameter("wp", [8, P, DIM], F16, isOutput=False)
    cos4 = nc.declare_dram_parameter("cos4", [P, L], F16, isOutput=False)
    sin4 = nc.declare_dram_parameter("sin4", [P, L], F16, isOutput=False)
    # sel128[k, cpar, m] = 1.0 where k == 32*(2*cpar + m//64): K=128 one-hot
    # matmul replicating softmax-sum rows (at partitions 0/32/64/96) onto
    # the [128, 512] head-pair layout
    sel = nc.declare_dram_parameter("sel", [P, 2, P], F16, isOutput=False)
    # bf16 output: halves the 4MB/core output DMA; host casts to fp32
    out = nc.declare_dram_parameter("out", [L, DIM], F16, isOutput=True)

    Exp = mybir.ActivationFunctionType.Exp
    MUL = mybir.AluOpType.mult
    ADD = mybir.AluOpType.add
    SUB = mybir.AluOpType.subtract

    with tile.TileContext(nc) as tc:
        with (
            tc.tile_pool(name="consts", bufs=1) as consts,
            tc.tile_pool(name="qk", bufs=1) as qkpool,
            tc.tile_pool(name="vmat", bufs=1) as vpool,
            tc.tile_pool(name="outTp", bufs=1) as opool,
            tc.tile_pool(name="small", bufs=1) as small,
            tc.tile_pool(name="xt", bufs=1) as xtp,
            tc.tile_pool(name="wqks", bufs=6) as wqks,
            tc.tile_pool(name="ropetmp", bufs=8) as rtmp,
            tc.tile_pool(name="wmat", bufs=8) as wmat,
            tc.tile_pool(name="wpmat", bufs=8) as wpmat,
            tc.tile_pool(name="probs", bufs=4) as probs,
            tc.tile_pool(name="stag", bufs=2) as stag,
            tc.tile_pool(name="rrec", bufs=4) as rrec,
            tc.tile_pool(name="y", bufs=2) as ypool,
            tc.tile_pool(name="yacc", bufs=1) as yaccp,
            tc.tile_pool(name="psB", bufs=1, space="PSUM") as psB,
            tc.tile_pool(name="psV", bufs=2, space="PSUM") as psV,
            tc.tile_pool(name="psSum", bufs=1, space="PSUM") as psSum,
            tc.tile_pool(name="psF", bufs=1, space="PSUM") as psF,
        ):
            # ---- startup DMAs: xt on sync ring, weights on scalar ring.
            # DMA is ~390 GB/s SHARED across rings; the first matmul (v0)
            # needs only xt0+wv0 and the dc-chain streams behind the xt
            # arrivals, so interleave per-chunk.
            xt_sb = xtp.tile([P, 8, L], F16, tag="xt")
            wv_t = []
            for dc in range(8):
                nc.sync.dma_start(xt_sb[:, dc, :], xT[ts(dc, P), :])
                w = wmat.tile([P, DIM], F16, tag="w", name=f"wv{dc}")
                nc.scalar.dma_start(w[:], wv[dc])
                wv_t.append(w)

            cos_sb = consts.tile([P, L], F16, tag="cos")
            sin_sb = consts.tile([P, L], F16, tag="sin")
            ones_c = consts.tile([P, 1], F16, tag="ones")
            sel_sb = small.tile([P, 2, P], F16, tag="sel")
            nc.scalar.dma_start(cos_sb[:], cos4[:])
            nc.scalar.dma_start(sin_sb[:], sin4[:])
            nc.scalar.dma_start(sel_sb[:], sel[:])
            nc.gpsimd.memset(ones_c[:], 1.0)

            q_sb = qkpool.tile([P, 8, L], F16, tag="q")
            k_sb = qkpool.tile([P, 8, L], F16, tag="k")
            v_sb = vpool.tile([P, 8, H, D], F16, tag="v")
            outT = opool.tile([P, 8, L], F16, tag="o")
            y_acc = yaccp.tile([P, 8, L], F32, tag="ya")

            def v_chunk_gen(tc_):
                """Generator: v for token chunk tc_ via psF halves."""
                for jh in (0, 1):
                    jsl = slice(512 * jh, 512 * jh + 512)
                    V = psF.tile([P, 512], F32, tag="pf", name="Vt")
                    for dc in range(8):
                        nc.tensor.matmul(
                            V[:],
                            lhsT=xt_sb[:, dc, ts(tc_, P)],
                            rhs=wv_t[dc][:, jsl],
                            start=(dc == 0), stop=(dc == 7),
                        )
                        if dc == 3:
                            yield 860 + _UNSAFE
                    with tc.high_priority(offset=30):
                        nc.vector.tensor_copy(
                            v_sb[:, tc_, 8 * jh:8 * jh + 8, :],
                            V[:].rearrange("p (h d) -> p h d", d=D),
                        )
                    yield 900

            # PSUM WAR deps are tile-granular: a tile being evacuated must
            # not receive new writes until the read completes, so ping-pong
            # needs SEPARATE tiles. Pre/tail units cycle their [128,512]
            # halves across four distinct pool slots (psF, psSum, psV x2).
            slot_state = [0]

            def _slot_tile(nm):
                i = slot_state[0] % 4
                slot_state[0] += 1
                if i == 0:
                    return psF.tile([P, 512], F32, tag="pf", name=nm)
                if i == 1:
                    return psSum.tile([P, 512], F32, tag="ps", name=nm)
                return psV.tile([P, 512], F32, tag="pv", name=nm)

            def pre_units(units):
                """Pre-attention chunk units, halves ping-ponged across
                the 4 pool slots: unit i+1's matmuls overlap unit i's
                evacs. c >= 16 -> v chunk c-16, else q/k chunk c.
                Returns epre tiles for q/k units."""
                ev = []
                for c in units:
                    if c >= 16:   # v chunk c-16
                        vc = c - 16
                        for jh in (0, 1):
                            jsl = slice(512 * jh, 512 * jh + 512)
                            V = _slot_tile("Vpre")
                            for dc in range(8):
                                nc.tensor.matmul(
                                    V[:],
                                    lhsT=xt_sb[:, dc, ts(vc, P)],
                                    rhs=wv_t[dc][:, jsl],
                                    start=(dc == 0), stop=(dc == 7),
                                )
                            nc.scalar.copy(
                                v_sb[:, vc, 8 * jh:8 * jh + 8, :],
                                V[:].rearrange("p (h d) -> p h d", d=D),
                            )
                    else:         # q/k chunk c
                        wt = wqk_pre.pop(c)
                        e = rtmp.tile([P, L], F16, tag="rt", name="epre")
                        for th in (0, 1):
                            tsl = slice(512 * th, 512 * th + 512)
                            S = _slot_tile("Spre")
                            for dc in range(8):
                                nc.tensor.matmul(
                                    S[:],
                                    lhsT=wt[:, ts(dc, P)],
                                    rhs=xt_sb[:, dc, tsl],
                                    start=(dc == 0), stop=(dc == 7),
                                )
                            nc.scalar.copy(e[:, tsl], S[:])
                        ev.append(e)
                return ev

            def v_rest(vcs):
                for tc_ in vcs:
                    yield from v_chunk_gen(tc_)

            def rope_pair(c, U, Lp):
                tgt = q_sb if c < 8 else k_sb
                ci = c if c < 8 else c - 8
                t1 = rtmp.tile([P, L], F16, tag="rt", name="t1")
                t2 = rtmp.tile([P, L], F16, tag="rt", name="t2")
                nc.vector.tensor_tensor(tgt[:, ci, :], U[:], cos_sb[:], MUL)
                nc.vector.tensor_tensor(t1[:], Lp[:], sin_sb[:], MUL)
                nc.vector.tensor_tensor(
                    tgt[:, ci, :], tgt[:, ci, :], t1[:], SUB)
                yield 250
                nc.vector.tensor_tensor(
                    tgt[:, ci + 1, :], Lp[:], cos_sb[:], MUL)
                nc.vector.tensor_tensor(t2[:], U[:], sin_sb[:], MUL)
                nc.vector.tensor_tensor(
                    tgt[:, ci + 1, :], tgt[:, ci + 1, :], t2[:], ADD)
                yield 250

            def qk_chunk_pair(c):
                """Generator producing q or k chunks (c, c+1) via psF
                halves, quarter-chunk burn granularity."""
                ev = []
                for cc in (c, c + 1):
                    wt = wqks.tile([P, DIM], F16, tag="wqk")
                    nc.sync.dma_start(wt[:], wqk[cc])
                    e = rtmp.tile([P, L], F16, tag="rt")
                    for th in (0, 1):
                        tsl = slice(512 * th, 512 * th + 512)
                        S = psF.tile([P, 512], F32, tag="pf", name="Sqk")
                        # mid-chain yields are UNSAFE (+_UNSAFE): the psF
                        # tile is half-written, so no other psF user may
                        # be issued until the chain resumes -- burn_one
                        # tracks this and to_safe() runs before the norm's
                        # Rs is issued.
                        for dc in range(8):
                            nc.tensor.matmul(
                                S[:],
                                lhsT=wt[:, ts(dc, P)],
                                rhs=xt_sb[:, dc, tsl],
                                start=(dc == 0), stop=(dc == 7),
                            )
                            if dc == 3:
                                yield 860 + _UNSAFE
                        with tc.high_priority(offset=30):
                            nc.vector.tensor_copy(e[:, tsl], S[:])
                        yield 900
                    ev.append(e)
                yield from rope_pair(c, ev[0], ev[1])



            def drain(feeder):
                if feeder is not None:
                    for _ in feeder:
                        pass

            def norm_evac(g, ih, pvAB, pvCD, st):
                """Generator: normalize+evacuate segment (g, ih) given its
                staged sums tile st. Spliced into the NEXT segment."""
                isl = slice(512 * ih, 512 * ih + 512)
                rs = []
                for cc in (2 * g, 2 * g + 1):
                    Rs = psF.tile([P, 512], F32, tag="pf", name="Rs")
                    nc.tensor.matmul(
                        Rs[:], lhsT=sel_sb[:, cc % 2, :], rhs=st[:],
                        start=True, stop=True,
                    )
                    Rr = rrec.tile([P, 512], F32, tag="rr")
                    with tc.high_priority(offset=30):
                        nc.vector.reciprocal_approx_fast(out=Rr[:],
                                                         in_=Rs[:])
                    rs.append(Rr)
                yield 440
                for cc, Rr, pvt in ((2 * g, rs[0], pvAB),
                                    (2 * g + 1, rs[1], pvCD)):
                    with tc.high_priority(offset=30):
                        nc.vector.tensor_tensor(
                            outT[:, cc, isl], pvt[:], Rr[:], MUL)
                yield 0

            wp_t = {}

            def proj_seg(s):
                """Generator: out-proj contribution of segment s=(g, ih):
                outT chunks (2g, 2g+1) x token chunks of q-half ih,
                accumulated into fp32 y_acc; g==3 adds y_acc and streams
                the output per (tc, eh) half on alternating rings."""
                g, ih = divmod(s, 2)
                ccs = (2 * g, 2 * g + 1)
                if ih == 0:
                    for cc in ccs:
                        w = wpmat.tile([P, DIM], F16, tag="wp",
                                       name=f"wp{cc}")
                        nc.sync.dma_start(w[:], wp[cc])
                        wp_t[cc] = w
                    yield 0
                # the post-stream tail (s == 7) cycles Y over the 4
                # distinct pool slots (psV/psSum free after attention),
                # so Yp matmuls overlap the DVE adds
                for u, tc_ in enumerate(range(4 * ih, 4 * ih + 4)):
                    ysb = (ypool.tile([P, DIM], F16, tag="y2", name="ysb")
                           if g == 3 else None)
                    for eh in (0, 1):
                        esl = slice(512 * eh, 512 * eh + 512)
                        if s == 7:
                            Y = _slot_tile("Ytl")[:]
                        else:
                            Y = psF.tile([P, 512], F32, tag="pf",
                                         name="Yp")[:]
                        for ix, cc in enumerate(ccs):
                            nc.tensor.matmul(
                                Y,
                                lhsT=outT[:, cc, ts(tc_, P)],
                                rhs=wp_t[cc][:, esl],
                                start=(ix == 0), stop=(ix == 1),
                            )
                        with tc.high_priority(offset=30):
                            if g == 0:
                                nc.vector.tensor_copy(
                                    y_acc[:, tc_, esl], Y)
                            elif g < 3:
                                nc.vector.tensor_tensor(
                                    y_acc[:, tc_, esl], y_acc[:, tc_, esl],
                                    Y, ADD)
                            else:
                                nc.vector.tensor_tensor(
                                    ysb[:, esl], Y, y_acc[:, tc_, esl],
                                    ADD)
                        if g == 3:
                            (nc.sync if eh == 0 else nc.scalar).dma_start(
                                out[ts(tc_, P), esl], ysb[:, esl])
                        yield 660

            def attention_stream(schedule):
                """Continuous stream over 8 segments (quad g, query-half
                ih). Per step: QK rounds + ONE fused exp; sums quad + PV
                trail by LAG. `schedule` is a list of (t_start, generator)
                feeder gates (must be sorted by t_start); feeder work is
                burned around the attention blocks to fill PE slack under
                the exp-latency step floor."""
                active = []
                pending = [None]
                unsafe = [False]   # active[0] suspended mid-psF-tile?

                def burn_one():
                    while True:
                        if not active:
                            return
                        w = next(active[0], _SENT)
                        if w is _SENT:
                            active.pop(0)
                            unsafe[0] = False
                        else:
                            unsafe[0] = w >= _UNSAFE
                            if (w % _UNSAFE) >= 300:
                                return

                def to_safe():
                    # resume active[0] past any half-written psF tile
                    # before another psF user (the norm's Rs) is issued
                    while unsafe[0] and active:
                        w = next(active[0], _SENT)
                        if w is _SENT:
                            active.pop(0)
                            unsafe[0] = False
                        else:
                            unsafe[0] = w >= _UNSAFE

                def drain_pending():
                    to_safe()
                    while pending[0] is not None:
                        if next(pending[0], _SENT) is _SENT:
                            pending[0] = None

                nseg = 8
                prbs = {}
                cur = {}
                for t in range(8 * nseg + LAG):
                    while schedule and schedule[0][0] <= t:
                        active.append(schedule.pop(0)[1])
                    s_pv, pj = divmod(t - LAG, 8)
                    # drain the previous segment's norm BEFORE this step's
                    # QK: with the high-priority st copy + to_safe, the Rs
                    # matmuls land in the exp(t-1) window instead of
                    # splitting the next score block
                    if 0 <= s_pv < nseg and pj == 0:
                        drain_pending()
                        cur['pvAB'] = psV.tile([P, 512], F32, tag="pv",
                                               name="pvAB")
                        cur['pvCD'] = psV.tile([P, 512], F32, tag="pv",
                                               name="pvCD")
                        cur['sum4'] = psSum.tile([P, 512], F32, tag="ps",
                                                 name="sum4")
                    burn_one()
                    s_qk, jc = divmod(t, 8)
                    if s_qk < nseg:
                        g, ih = divmod(s_qk, 2)
                        isl = slice(512 * ih, 512 * ih + 512)
                        # two per-step score tiles with their own 1-buf
                        # rings: SAB(t+1) only waits exp_A(t), halving the
                        # psB turnaround vs one fused 4-bank tile
                        SAB = psB.tile([P, L], F32, tag="sa", name="SAB")
                        SCD = psB.tile([P, L], F32, tag="sc", name="SCD")
                        s_of = {0: (SAB, 0), 1: (SAB, 512),
                                2: (SCD, 0), 3: (SCD, 512)}
                        for lo in (0, 1):   # up halves then lo halves
                            for j in range(4):
                                St, co = s_of[j]
                                psl = slice(32 * j, 32 * j + 32)
                                nc.tensor.matmul(
                                    St[:, co:co + 512],
                                    lhsT=k_sb[psl, 2 * g + lo, ts(jc, P)],
                                    rhs=q_sb[psl, 2 * g + lo, isl],
                                    start=(lo == 0), stop=(lo == 1),
                                    tile_position=(32 * j, 0),
                                )
                        prb = probs.tile([P, 2, L], F16, tag="pr")
                        nc.scalar.activation(prb[:, 0, :], SAB[:], Exp,
                                             scale=0.125)
                        nc.scalar.activation(prb[:, 1, :], SCD[:], Exp,
                                             scale=0.125)
                        prbs[t] = prb
                    burn_one()
                    if 0 <= s_pv < nseg:
                        g, ih = divmod(s_pv, 2)
                        heads = [4 * g + j for j in range(4)]
                        prb = prbs.pop(t - LAG)
                        p_of = {0: (0, 0), 1: (0, 512),
                                2: (1, 0), 3: (1, 512)}
                        # softmax-sum quad first: 4 col-tiled M=1 matmuls
                        # issued back-to-back run concurrently
                        for j in range(4):
                            hf, co = p_of[j]
                            nc.tensor.matmul(
                                cur['sum4'][32 * j:32 * j + 1, :],
                                lhsT=ones_c[:],
                                rhs=prb[:, hf, co:co + 512],
                                start=(pj == 0), stop=(pj == 7),
                                tile_position=(0, 32 * j),
                                skip_group_check=True,
                            )
                        for j in range(4):
                            hf, co = p_of[j]
                            pvt = cur['pvAB'] if j < 2 else cur['pvCD']
                            ro = (j % 2) * D
                            nc.tensor.matmul(
                                pvt[ro:ro + D, :],
                                lhsT=v_sb[:, pj, heads[j], :],
                                rhs=prb[:, hf, co:co + 512],
                                start=(pj == 0), stop=(pj == 7),
                                tile_position=(0, ro),
                                skip_group_check=True,
                            )
                        if pj == 7:
                            st = stag.tile([P, 512], F16, tag="st",
                                           name="st")
                            # high priority: jump the DVE queue so the
                            # next boundary's Rs never waits on this copy
                            with tc.high_priority(offset=40):
                                nc.vector.tensor_copy(st[:], cur['sum4'][:])
                            pending[0] = norm_evac(
                                g, ih, cur['pvAB'], cur['pvCD'], st)
                # retire the last segment's normalization, then leftovers.
                # Gates > loop-end (e.g. the final proj) are appended only
                # AFTER the drain: a reader of outT issued before the norm
                # mult would see stale data (WAR -- dep tracking orders by
                # issue order).
                drain_pending()
                while schedule:
                    active.append(schedule.pop(0)[1])
                while active:
                    burn_one()

            # ---------------- pipeline ----------------
            # pre-attention: v chunks 0-5 + q/k chunks 0,1 (q) and 8,9 (k).
            # The pre window is DMA-paced early, so the extra v chunks ride
            # for ~free and the in-stream feeder load drops to what fits
            # under the 2.8us exp-floor slack. wqk chunks prefetched so no
            # unit waits on a just-in-time weight DMA.
            wqk_pre = {}
            for c in (0, 1, 8, 9):
                wt0 = wqks.tile([P, DIM], F16, tag="wqk", name=f"wqkp{c}")
                nc.sync.dma_start(wt0[:], wqk[c])
                wqk_pre[c] = wt0
            ev_q = pre_units([16, 17, 0, 1])          # v0, v1, q0, q1
            rq = rope_pair(0, ev_q[0], ev_q[1])
            next(rq, None)
            ev_k = pre_units([18, 8, 9])              # v2, k0, k1
            drain(rq)
            rk = rope_pair(8, ev_k[0], ev_k[1])
            next(rk, None)
            pre_units([19, 20, 21])                   # v3, v4, v5
            drain(rk)

            # qk pair c is needed by segment (c % 8) // 2 * 2 (+1), i.e.
            # q/k quad g by t = 16g; gates spread each ~7us pair so the
            # per-step feeder load stays under the exp-floor slack.
            schedule = [
                (0, v_rest(range(6, 8))),
                (0, qk_chunk_pair(2)),        # q chunks 2,3 (need t=16)
                (5, qk_chunk_pair(10)),       # k chunks 2,3 (need t=16)
                (11, proj_seg(0)),
                (18, qk_chunk_pair(4)),       # q chunks 4,5 (need t=32)
                (19, proj_seg(1)),
                (25, qk_chunk_pair(12)),      # k chunks 4,5 (need t=32)
                (27, proj_seg(2)),
                (35, proj_seg(3)),
                (36, qk_chunk_pair(6)),       # q chunks 6,7 (need t=48)
                (42, qk_chunk_pair(14)),      # k chunks 6,7 (need t=48)
                (43, proj_seg(4)),
                (51, proj_seg(5)),
                (59, proj_seg(6)),
                (99, proj_seg(7)),   # post-loop: after the final norm drain
            ]
            attention_stream(schedule)

    nc.compile()
    return nc


_SENT = object()
_UNSAFE = 10000


def _qk_perm():
    """Column permutation for q (or k) weights: chunk 2g = upper halves
    (d 0:32) of heads 4g..4g+3, chunk 2g+1 = lower halves."""
    perm = []
    for g in range(4):
        for d0 in (0, 32):
            for j in range(4):
                h = 4 * g + j
                perm.extend(h * D + d for d in range(d0, d0 + 32))
    return np.asarray(perm)


def prep_shards(hidden_states, cos, sin, w_qkv, b_qkv, w_proj, b_proj,
                cu_seqlens=None):
    """Build the per-core input maps (host-side, numpy)."""
    perm = _qk_perm()
    wq = w_qkv[:, :DIM][:, perm]
    wk = w_qkv[:, DIM:2 * DIM][:, perm]
    wqk_cols = np.concatenate([wq, wk], axis=1)            # [1024, 2048]
    # Wqk[c, dp, dc*128 + j] = wqk_cols[dc*128 + dp, c*128 + j]
    Wqk = np.ascontiguousarray(
        wqk_cols.reshape(8, P, 16, P).transpose(2, 1, 0, 3).reshape(16, P, DIM)
    ).astype(ml_dtypes.bfloat16)
    Wv = np.ascontiguousarray(
        w_qkv[:, 2 * DIM:].reshape(8, P, DIM)).astype(ml_dtypes.bfloat16)
    Wp = np.ascontiguousarray(
        w_proj.reshape(8, P, DIM)).astype(ml_dtypes.bfloat16)

    in_maps = []
    for i in range(NCORES):
        sl = slice(i * L, (i + 1) * L)
        xT = np.ascontiguousarray(
            hidden_states[sl].T).astype(ml_dtypes.bfloat16)
        cosT = cos[sl, :D // 2].T.astype(np.float32)       # [32, 1024]
        sinT = sin[sl, :D // 2].T.astype(np.float32)
        cos4 = np.ascontiguousarray(
            np.tile(cosT, (4, 1))).astype(ml_dtypes.bfloat16)
        sin4 = np.ascontiguousarray(
            np.tile(sinT, (4, 1))).astype(ml_dtypes.bfloat16)
        in_maps.append({
            "xT": xT, "wqk": Wqk, "wv": Wv, "wp": Wp,
            "cos4": cos4, "sin4": sin4, "sel": _sel_mat(),
        })
    return in_maps


def _sel_mat():
    sel = np.zeros((P, 2, P), ml_dtypes.bfloat16)
    for cpar in range(2):
        for m in range(P):
            sel[32 * (2 * cpar + m // D), cpar, m] = 1.0
    return sel


_NC_CACHE = {}


def kernel(hidden_states, cos, sin, w_qkv, b_qkv, w_proj, b_proj,
           cu_seqlens=None, **_unused):
    hidden_states = np.asarray(hidden_states)
    assert hidden_states.shape == (NCORES * L, DIM)

    from concourse.bass_utils import run_bass_kernel_spmd

    if "nc" not in _NC_CACHE:
        _NC_CACHE["nc"] = build_nc()
    nc = _NC_CACHE["nc"]

    in_maps = prep_shards(np.asarray(hidden_states), np.asarray(cos),
                          np.asarray(sin), np.asarray(w_qkv),
                          np.asarray(b_qkv), np.asarray(w_proj),
                          np.asarray(b_proj))
    res = run_bass_kernel_spmd(nc, in_maps, core_ids=list(range(NCORES)))
    out = np.concatenate([res.results[i]["out"] for i in range(NCORES)],
                         axis=0)
    return out.astype(np.float32)


# revision 45
# speedup vs baseline: 1.0016x; 1.0016x over previous
"""Trainium2 Bass kernel for AIMv2FlashAttention2 (packed varlen attention).

Problem: hidden [8192, 1024] = 8 packed sequences x 1024 tokens, dim=1024,
16 heads x 64 head_dim. qkv proj + RoPE (rotate-half) + block-diagonal
softmax attention + out proj.

Strategy: pure data parallelism -- attention is block-diagonal per sequence,
so each of the 8 NeuronCores processes one full sequence locally with
replicated weights. Zero collectives.

v10 structure (exp-floor-paced stream; fused per-step exp):
  - scores for a step live in ONE [128, 2, 1024] PSUM tile (4 banks,
    bufs=1): 8 K=32 QK matmuls on 4 PE row-groups, then a SINGLE
    ScalarE exp over all 2048 columns ((2048+352)/1.2 = 2.0us vs
    2x1.15 split) -- ScalarE stops being the step pacer.
  - per step: QK (2 rounds of 4 row-tiled matmuls) -> fused exp ->
    softmax-sum quad (4 col-tiled M=1 matmuls, concurrent) -> PV
    (2 rounds of 2 col-tiled M=64), PV/sums trail QK by LAG=2 steps.
  - feeder work (qkv chunks, v chunks, per-seg out-proj, norm
    broadcasts) is spliced between attention blocks by a weighted burn
    (2 sites/step); qk feeders yield at quarter-chunk granularity so
    no step absorbs more than ~1us of feeder work (the old full-chunk
    bursts made feeder steps 2x slower than the exp floor).
  - out-proj restructured per segment: as soon as segment (g, ih)
    normalizes, its outT chunk pair (2g, 2g+1) is projected for the 4
    token chunks of q-half ih and accumulated into fp32 y_acc; the
    g==3 contribution adds y_acc and streams the output out per
    (tc, eh) half on alternating HWDGE rings. Only segment (3,1)'s
    projection trails the stream (~7us tail).
  - normalization: sums staged to SBUF (one wide DVE copy, high
    priority), broadcast via one-hot K=128 matmul, reciprocal, single
    multiply evacuates PSUM->outT; drained AFTER the next segment's QK
    block so the Rs matmul never head-of-line blocks the QK/exp pacer.
  - PSUM: scores 1x[128,2048] (4 banks), pv 2x[128,512], sums 1 bank,
    feeder scratch 1 bank.
  - bf16 output (host casts back to fp32): halves the output DMA.
"""

import numpy as np
import ml_dtypes

import concourse.bass as bass
import concourse.bacc as bacc
import concourse.mybir as mybir
import concourse.tile as tile
from concourse.bass import ts

F32 = mybir.dt.float32
F16 = mybir.dt.bfloat16

P = 128
L = 1024          # tokens per sequence / core
DIM = 1024
H = 16            # heads
D = 64            # head dim
NCORES = 8
LAG = 2           # PV trails QK by this many jc steps


def build_nc(dbg=False):
    nc = bacc.Bacc(None)

    xT = nc.declare_dram_parameter("xT", [DIM, L], F16, isOutput=False)
    wqk = nc.declare_dram_parameter("wqk", [16, P, DIM], F16, isOutput=False)
    wv = nc.declare_dram_parameter("wv", [8, P, DIM], F16, isOutput=False)
    wp = nc.declare_dram_parameter("wp", [8, P, DIM], F16, isOutput=False)
    cos4 = nc.declare_dram_parameter("cos4", [P, L], F16, isOutput=False)
    sin4 = nc.declare_dram_parameter("sin4", [P, L], F16, isOutput=False)
    # sel128[k, cpar, m] = 1.0 where k == 32*(2*cpar + m//64): K=128 one-hot
    # matmul replicating softmax-sum rows (at partitions 0/32/64/96) onto
    # the [128, 512] head-pair layout
    sel = nc.declare_dram_parameter("sel", [P, 2, P], F16, isOutput=False)
    # bf16 output: halves the 4MB/core output DMA; host casts to fp32
    out = nc.declare_dram_parameter("out", [L, DIM], F16, isOutput=True)

    Exp = mybir.ActivationFunctionType.Exp
    MUL = mybir.AluOpType.mult
    ADD = mybir.AluOpType.add
    SUB = mybir.AluOpType.subtract

    with tile.TileContext(nc) as tc:
        with (
            tc.tile_pool(name="consts", bufs=1) as consts,
            tc.tile_pool(name="qk", bufs=1) as qkpool,
            tc.tile_pool(name="vmat", bufs=1) as vpool,
            tc.tile_pool(name="outTp", bufs=1) as opool,
            tc.tile_pool(name="small", bufs=1) as small,
            tc.tile_pool(name="xt", bufs=1) as xtp,
            tc.tile_pool(name="wqks", bufs=6) as wqks,
            tc.tile_pool(name="ropetmp", bufs=8) as rtmp,
            tc.tile_pool(name="wmat", bufs=8) as wmat,
            tc.tile_pool(name="wpmat", bufs=8) as wpmat,
            tc.tile_pool(name="probs", bufs=4) as probs,
            tc.tile_pool(name="stag", bufs=2) as stag,
            tc.tile_pool(name="rrec", bufs=4) as rrec,
            tc.tile_pool(name="y", bufs=2) as ypool,
            tc.tile_pool(name="yacc", bufs=1) as yaccp,
            tc.tile_pool(name="psB", bufs=1, space="PSUM") as psB,
            tc.tile_pool(name="psV", bufs=2, space="PSUM") as psV,
            tc.tile_pool(name="psSum", bufs=1, space="PSUM") as psSum,
            tc.tile_pool(name="psF", bufs=1, space="PSUM") as psF,
        ):
            # ---- startup DMAs: xt on sync ring, weights on scalar ring.
            # DMA is ~390 GB/s SHARED across rings; the first matmul (v0)
            # needs only xt0+wv0 and the dc-chain streams behind the xt
            # arrivals, so interleave per-chunk.
            xt_sb = xtp.tile([P, 8, L], F16, tag="xt")
            wv_t = []
            for dc in range(8):
                nc.sync.dma_start(xt_sb[:, dc, :], xT[ts(dc, P), :])
                w = wmat.tile([P, DIM], F16, tag="w", name=f"wv{dc}")
                nc.scalar.dma_start(w[:], wv[dc])
                wv_t.append(w)

            cos_sb = consts.tile([P, L], F16, tag="cos")
            sin_sb = consts.tile([P, L], F16, tag="sin")
            ones_c = consts.tile([P, 1], F16, tag="ones")
            sel_sb = small.tile([P, 2, P], F16, tag="sel")
            nc.scalar.dma_start(cos_sb[:], cos4[:])
            nc.scalar.dma_start(sin_sb[:], sin4[:])
            nc.scalar.dma_start(sel_sb[:], sel[:])
            nc.gpsimd.memset(ones_c[:], 1.0)

            q_sb = qkpool.tile([P, 8, L], F16, tag="q")
            k_sb = qkpool.tile([P, 8, L], F16, tag="k")
            v_sb = vpool.tile([P, 8, H, D], F16, tag="v")
            outT = opool.tile([P, 8, L], F16, tag="o")
            y_acc = yaccp.tile([P, 8, L], F32, tag="ya")

            def v_chunk_gen(tc_):
                """Generator: v for token chunk tc_ via psF halves."""
                for jh in (0, 1):
                    jsl = slice(512 * jh, 512 * jh + 512)
                    V = psF.tile([P, 512], F32, tag="pf", name="Vt")
                    for dc in range(8):
                        nc.tensor.matmul(
                            V[:],
                            lhsT=xt_sb[:, dc, ts(tc_, P)],
                            rhs=wv_t[dc][:, jsl],
                            start=(dc == 0), stop=(dc == 7),
                        )
                        if dc == 3:
                            yield 860 + _UNSAFE
                    with tc.high_priority(offset=30):
                        nc.vector.tensor_copy(
                            v_sb[:, tc_, 8 * jh:8 * jh + 8, :],
                            V[:].rearrange("p (h d) -> p h d", d=D),
                        )
                    yield 900

            # PSUM WAR deps are tile-granular: a tile being evacuated must
            # not receive new writes until the read completes, so ping-pong
            # needs SEPARATE tiles. Pre/tail units cycle their [128,512]
            # halves across four distinct pool slots (psF, psSum, psV x2).
            slot_state = [0]

            def _slot_tile(nm):
                i = slot_state[0] % 4
                slot_state[0] += 1
                if i == 0:
                    return psF.tile([P, 512], F32, tag="pf", name=nm)
                if i == 1:
                    return psSum.tile([P, 512], F32, tag="ps", name=nm)
                return psV.tile([P, 512], F32, tag="pv", name=nm)

            def pre_units(units):
                """Pre-attention chunk units, halves ping-ponged across
                the 4 pool slots: unit i+1's matmuls overlap unit i's
                evacs. c >= 16 -> v chunk c-16, else q/k chunk c.
                Returns epre tiles for q/k units."""
                ev = []
                for c in units:
                    if c >= 16:   # v chunk c-16
                        vc = c - 16
                        for jh in (0, 1):
                            jsl = slice(512 * jh, 512 * jh + 512)
                            V = _slot_tile("Vpre")
                            for dc in range(8):
                                nc.tensor.matmul(
                                    V[:],
                                    lhsT=xt_sb[:, dc, ts(vc, P)],
                                    rhs=wv_t[dc][:, jsl],
                                    start=(dc == 0), stop=(dc == 7),
                                )
                            nc.scalar.copy(
                                v_sb[:, vc, 8 * jh:8 * jh + 8, :],
                                V[:].rearrange("p (h d) -> p h d", d=D),
                            )
                    else:         # q/k chunk c
                        wt = wqk_pre.pop(c)
                        e = rtmp.tile([P, L], F16, tag="rt", name="epre")
                        for th in (0, 1):
                            tsl = slice(512 * th, 512 * th + 512)
                            S = _slot_tile("Spre")
                            for dc in range(8):
                                nc.tensor.matmul(
                                    S[:],
                                    lhsT=wt[:, ts(dc, P)],
                                    rhs=xt_sb[:, dc, tsl],
                                    start=(dc == 0), stop=(dc == 7),
                                )
                            nc.scalar.copy(e[:, tsl], S[:])
                        ev.append(e)
                return ev

            def v_rest(vcs):
                for tc_ in vcs:
                    yield from v_chunk_gen(tc_)

            def rope_pair(c, U, Lp):
                tgt = q_sb if c < 8 else k_sb
                ci = c if c < 8 else c - 8
                t1 = rtmp.tile([P, L], F16, tag="rt", name="t1")
                t2 = rtmp.tile([P, L], F16, tag="rt", name="t2")
                nc.vector.tensor_tensor(tgt[:, ci, :], U[:], cos_sb[:], MUL)
                nc.vector.tensor_tensor(t1[:], Lp[:], sin_sb[:], MUL)
                nc.vector.tensor_tensor(
                    tgt[:, ci, :], tgt[:, ci, :], t1[:], SUB)
                yield 250
                nc.vector.tensor_tensor(
                    tgt[:, ci + 1, :], Lp[:], cos_sb[:], MUL)
                nc.vector.tensor_tensor(t2[:], U[:], sin_sb[:], MUL)
                nc.vector.tensor_tensor(
                    tgt[:, ci + 1, :], tgt[:, ci + 1, :], t2[:], ADD)
                yield 250

            def qk_chunk_pair(c):
                """Generator producing q or k chunks (c, c+1) via psF
                halves, quarter-chunk burn granularity."""
                ev = []
                for cc in (c, c + 1):
                    wt = wqks.tile([P, DIM], F16, tag="wqk")
                    nc.sync.dma_start(wt[:], wqk[cc])
                    e = rtmp.tile([P, L], F16, tag="rt")
                    for th in (0, 1):
                        tsl = slice(512 * th, 512 * th + 512)
                        S = psF.tile([P, 512], F32, tag="pf", name="Sqk")
                        # mid-chain yields are UNSAFE (+_UNSAFE): the psF
                        # tile is half-written, so no other psF user may
                        # be issued until the chain resumes -- burn_one
                        # tracks this and to_safe() runs before the norm's
                        # Rs is issued.
                        for dc in range(8):
                            nc.tensor.matmul(
                                S[:],
                                lhsT=wt[:, ts(dc, P)],
                                rhs=xt_sb[:, dc, tsl],
                                start=(dc == 0), stop=(dc == 7),
                            )
                            if dc == 3:
                                yield 860 + _UNSAFE
                        with tc.high_priority(offset=30):
                            nc.vector.tensor_copy(e[:, tsl], S[:])
                        yield 900
                    ev.append(e)
                yield from rope_pair(c, ev[0], ev[1])



            def drain(feeder):
                if feeder is not None:
                    for _ in feeder:
                        pass

            def norm_evac(g, ih, pvAB, pvCD, st):
                """Generator: normalize+evacuate segment (g, ih) given its
                staged sums tile st. Spliced into the NEXT segment."""
                isl = slice(512 * ih, 512 * ih + 512)
                rs = []
                for cc in (2 * g, 2 * g + 1):
                    Rs = psF.tile([P, 512], F32, tag="pf", name="Rs")
                    nc.tensor.matmul(
                        Rs[:], lhsT=sel_sb[:, cc % 2, :], rhs=st[:],
                        start=True, stop=True,
                    )
                    Rr = rrec.tile([P, 512], F32, tag="rr")
                    with tc.high_priority(offset=30):
                        nc.vector.reciprocal_approx_fast(out=Rr[:],
                                                         in_=Rs[:])
                    rs.append(Rr)
                yield 440
                for cc, Rr, pvt in ((2 * g, rs[0], pvAB),
                                    (2 * g + 1, rs[1], pvCD)):
                    with tc.high_priority(offset=30):
                        nc.vector.tensor_tensor(
                            outT[:, cc, isl], pvt[:], Rr[:], MUL)
                yield 0

            wp_t = {}

            def proj_quad(ph, ih):
                """Generator: out-proj of outT chunk quad ccs=4ph..4ph+3
                for q-half ih's 4 token chunks. Two phases per (tc, eh):
                ph0 initializes fp32 y_acc, ph1 adds it and streams the
                output half out on alternating rings. One DVE op per unit
                (vs one per chunk-pair) keeps the psF ring chain short."""
                ccs = range(4 * ph, 4 * ph + 4)
                if ih == 0:
                    for cc in ccs:
                        w = wpmat.tile([P, DIM], F16, tag="wp",
                                       name=f"wp{cc}")
                        nc.sync.dma_start(w[:], wp[cc])
                        wp_t[cc] = w
                    yield 0
                tail = (ph == 1 and ih == 1)
                for tc_ in range(4 * ih, 4 * ih + 4):
                    ysb = (ypool.tile([P, DIM], F16, tag="y2", name="ysb")
                           if ph == 1 else None)
                    for eh in (0, 1):
                        esl = slice(512 * eh, 512 * eh + 512)
                        if tail:
                            Y = _slot_tile("Ytl")[:]
                        else:
                            Y = psF.tile([P, 512], F32, tag="pf",
                                         name="Yp")[:]
                        for ix, cc in enumerate(ccs):
                            nc.tensor.matmul(
                                Y,
                                lhsT=outT[:, cc, ts(tc_, P)],
                                rhs=wp_t[cc][:, esl],
                                start=(ix == 0), stop=(ix == 3),
                            )
                        with tc.high_priority(offset=30):
                            if ph == 0:
                                nc.vector.tensor_copy(
                                    y_acc[:, tc_, esl], Y)
                            else:
                                nc.vector.tensor_tensor(
                                    ysb[:, esl], Y, y_acc[:, tc_, esl],
                                    ADD)
                        if ph == 1:
                            (nc.sync if eh == 0 else nc.scalar).dma_start(
                                out[ts(tc_, P), esl], ysb[:, esl])
                        yield 1100

            def attention_stream(schedule):
                """Continuous stream over 8 segments (quad g, query-half
                ih). Per step: QK rounds + ONE fused exp; sums quad + PV
                trail by LAG. `schedule` is a list of (t_start, generator)
                feeder gates (must be sorted by t_start); feeder work is
                burned around the attention blocks to fill PE slack under
                the exp-latency step floor."""
                active = []
                pending = [None]
                unsafe = [False]   # active[0] suspended mid-psF-tile?

                def burn_one():
                    while True:
                        if not active:
                            return
                        w = next(active[0], _SENT)
                        if w is _SENT:
                            active.pop(0)
                            unsafe[0] = False
                        else:
                            unsafe[0] = w >= _UNSAFE
                            if (w % _UNSAFE) >= 300:
                                return

                def to_safe():
                    # resume active[0] past any half-written psF tile
                    # before another psF user (the norm's Rs) is issued
                    while unsafe[0] and active:
                        w = next(active[0], _SENT)
                        if w is _SENT:
                            active.pop(0)
                            unsafe[0] = False
                        else:
                            unsafe[0] = w >= _UNSAFE

                def drain_pending():
                    to_safe()
                    while pending[0] is not None:
                        if next(pending[0], _SENT) is _SENT:
                            pending[0] = None

                nseg = 8
                prbs = {}
                cur = {}
                for t in range(8 * nseg + LAG):
                    while schedule and schedule[0][0] <= t:
                        active.append(schedule.pop(0)[1])
                    s_pv, pj = divmod(t - LAG, 8)
                    # drain the previous segment's norm BEFORE this step's
                    # QK: with the high-priority st copy + to_safe, the Rs
                    # matmuls land in the exp(t-1) window instead of
                    # splitting the next score block
                    if 0 <= s_pv < nseg and pj == 0:
                        drain_pending()
                        cur['pvAB'] = psV.tile([P, 512], F32, tag="pv",
                                               name="pvAB")
                        cur['pvCD'] = psV.tile([P, 512], F32, tag="pv",
                                               name="pvCD")
                        cur['sum4'] = psSum.tile([P, 512], F32, tag="ps",
                                                 name="sum4")
                    burn_one()
                    s_qk, jc = divmod(t, 8)
                    if s_qk < nseg:
                        g, ih = divmod(s_qk, 2)
                        isl = slice(512 * ih, 512 * ih + 512)
                        # two per-step score tiles with their own 1-buf
                        # rings: SAB(t+1) only waits exp_A(t), halving the
                        # psB turnaround vs one fused 4-bank tile
                        SAB = psB.tile([P, L], F32, tag="sa", name="SAB")
                        SCD = psB.tile([P, L], F32, tag="sc", name="SCD")
                        s_of = {0: (SAB, 0), 1: (SAB, 512),
                                2: (SCD, 0), 3: (SCD, 512)}
                        for lo in (0, 1):   # up halves then lo halves
                            for j in range(4):
                                St, co = s_of[j]
                                psl = slice(32 * j, 32 * j + 32)
                                nc.tensor.matmul(
                                    St[:, co:co + 512],
                                    lhsT=k_sb[psl, 2 * g + lo, ts(jc, P)],
                                    rhs=q_sb[psl, 2 * g + lo, isl],
                                    start=(lo == 0), stop=(lo == 1),
                                    tile_position=(32 * j, 0),
                                )
                        prb = probs.tile([P, 2, L], F16, tag="pr")
                        nc.scalar.activation(prb[:, 0, :], SAB[:], Exp,
                                             scale=0.125)
                        nc.scalar.activation(prb[:, 1, :], SCD[:], Exp,
                                             scale=0.125)
                        prbs[t] = prb
                    burn_one()
                    if 0 <= s_pv < nseg:
                        g, ih = divmod(s_pv, 2)
                        heads = [4 * g + j for j in range(4)]
                        prb = prbs.pop(t - LAG)
                        p_of = {0: (0, 0), 1: (0, 512),
                                2: (1, 0), 3: (1, 512)}
                        # softmax-sum quad first: 4 col-tiled M=1 matmuls
                        # issued back-to-back run concurrently
                        for j in range(4):
                            hf, co = p_of[j]
                            nc.tensor.matmul(
                                cur['sum4'][32 * j:32 * j + 1, :],
                                lhsT=ones_c[:],
                                rhs=prb[:, hf, co:co + 512],
                                start=(pj == 0), stop=(pj == 7),
                                tile_position=(0, 32 * j),
                                skip_group_check=True,
                            )
                        for j in range(4):
                            hf, co = p_of[j]
                            pvt = cur['pvAB'] if j < 2 else cur['pvCD']
                            ro = (j % 2) * D
                            nc.tensor.matmul(
                                pvt[ro:ro + D, :],
                                lhsT=v_sb[:, pj, heads[j], :],
                                rhs=prb[:, hf, co:co + 512],
                                start=(pj == 0), stop=(pj == 7),
                                tile_position=(0, ro),
                                skip_group_check=True,
                            )
                        if pj == 7:
                            st = stag.tile([P, 512], F16, tag="st",
                                           name="st")
                            # high priority: jump the DVE queue so the
                            # next boundary's Rs never waits on this copy
                            with tc.high_priority(offset=40):
                                nc.vector.tensor_copy(st[:], cur['sum4'][:])
                            pending[0] = norm_evac(
                                g, ih, cur['pvAB'], cur['pvCD'], st)
                # retire the last segment's normalization, then leftovers.
                # Gates > loop-end (e.g. the final proj) are appended only
                # AFTER the drain: a reader of outT issued before the norm
                # mult would see stale data (WAR -- dep tracking orders by
                # issue order).
                drain_pending()
                while schedule:
                    active.append(schedule.pop(0)[1])
                while active:
                    burn_one()

            # ---------------- pipeline ----------------
            # pre-attention: v chunks 0-5 + q/k chunks 0,1 (q) and 8,9 (k).
            # The pre window is DMA-paced early, so the extra v chunks ride
            # for ~free and the in-stream feeder load drops to what fits
            # under the 2.8us exp-floor slack. wqk chunks prefetched so no
            # unit waits on a just-in-time weight DMA.
            wqk_pre = {}
            for c in (0, 1, 8, 9):
                wt0 = wqks.tile([P, DIM], F16, tag="wqk", name=f"wqkp{c}")
                nc.sync.dma_start(wt0[:], wqk[c])
                wqk_pre[c] = wt0
            ev_q = pre_units([16, 17, 0, 1])          # v0, v1, q0, q1
            rq = rope_pair(0, ev_q[0], ev_q[1])
            next(rq, None)
            ev_k = pre_units([18, 8, 9])              # v2, k0, k1
            drain(rq)
            rk = rope_pair(8, ev_k[0], ev_k[1])
            next(rk, None)
            pre_units([19, 20, 21])                   # v3, v4, v5
            drain(rk)

            # qk pair c is needed by segment (c % 8) // 2 * 2 (+1), i.e.
            # q/k quad g by t = 16g; gates spread each ~7us pair so the
            # per-step feeder load stays under the exp-floor slack.
            schedule = [
                (0, v_rest(range(6, 8))),
                (0, qk_chunk_pair(2)),        # q chunks 2,3 (need t=16)
                (5, qk_chunk_pair(10)),       # k chunks 2,3 (need t=16)
                (18, qk_chunk_pair(4)),       # q chunks 4,5 (need t=32)
                (25, qk_chunk_pair(12)),      # k chunks 4,5 (need t=32)
                (27, proj_quad(0, 0)),        # ccs 0-3, tcs 0-3
                (35, proj_quad(0, 1)),        # ccs 0-3, tcs 4-7
                (36, qk_chunk_pair(6)),       # q chunks 6,7 (need t=48)
                (42, qk_chunk_pair(14)),      # k chunks 6,7 (need t=48)
                (59, proj_quad(1, 0)),        # ccs 4-7, tcs 0-3
                (99, proj_quad(1, 1)),   # tail: after the final norm drain
            ]
            attention_stream(schedule)

    nc.compile()
    return nc


_SENT = object()
_UNSAFE = 10000


def _qk_perm():
    """Column permutation for q (or k) weights: chunk 2g = upper halves
    (d 0:32) of heads 4g..4g+3, chunk 2g+1 = lower halves."""
    perm = []
    for g in range(4):
        for d0 in (0, 32):
            for j in range(4):
                h = 4 * g + j
                perm.extend(h * D + d for d in range(d0, d0 + 32))
    return np.asarray(perm)


def prep_shards(hidden_states, cos, sin, w_qkv, b_qkv, w_proj, b_proj,
                cu_seqlens=None):
    """Build the per-core input maps (host-side, numpy)."""
    perm = _qk_perm()
    wq = w_qkv[:, :DIM][:, perm]
    wk = w_qkv[:, DIM:2 * DIM][:, perm]
    wqk_cols = np.concatenate([wq, wk], axis=1)            # [1024, 2048]
    # Wqk[c, dp, dc*128 + j] = wqk_cols[dc*128 + dp, c*128 + j]
    Wqk = np.ascontiguousarray(
        wqk_cols.reshape(8, P, 16, P).transpose(2, 1, 0, 3).reshape(16, P, DIM)
    ).astype(ml_dtypes.bfloat16)
    Wv = np.ascontiguousarray(
        w_qkv[:, 2 * DIM:].reshape(8, P, DIM)).astype(ml_dtypes.bfloat16)
    Wp = np.ascontiguousarray(
        w_proj.reshape(8, P, DIM)).astype(ml_dtypes.bfloat16)

    in_maps = []
    for i in range(NCORES):
        sl = slice(i * L, (i + 1) * L)
        xT = np.ascontiguousarray(
            hidden_states[sl].T).astype(ml_dtypes.bfloat16)
        cosT = cos[sl, :D // 2].T.astype(np.float32)       # [32, 1024]
        sinT = sin[sl, :D // 2].T.astype(np.float32)
        cos4 = np.ascontiguousarray(
            np.tile(cosT, (4, 1))).astype(ml_dtypes.bfloat16)
        sin4 = np.ascontiguousarray(
            np.tile(sinT, (4, 1))).astype(ml_dtypes.bfloat16)
        in_maps.append({
            "xT": xT, "wqk": Wqk, "wv": Wv, "wp": Wp,
            "cos4": cos4, "sin4": sin4, "sel": _sel_mat(),
        })
    return in_maps


def _sel_mat():
    sel = np.zeros((P, 2, P), ml_dtypes.bfloat16)
    for cpar in range(2):
        for m in range(P):
            sel[32 * (2 * cpar + m // D), cpar, m] = 1.0
    return sel


_NC_CACHE = {}


def kernel(hidden_states, cos, sin, w_qkv, b_qkv, w_proj, b_proj,
           cu_seqlens=None, **_unused):
    hidden_states = np.asarray(hidden_states)
    assert hidden_states.shape == (NCORES * L, DIM)

    from concourse.bass_utils import run_bass_kernel_spmd

    if "nc" not in _NC_CACHE:
        _NC_CACHE["nc"] = build_nc()
    nc = _NC_CACHE["nc"]

    in_maps = prep_shards(np.asarray(hidden_states), np.asarray(cos),
                          np.asarray(sin), np.asarray(w_qkv),
                          np.asarray(b_qkv), np.asarray(w_proj),
                          np.asarray(b_proj))
    res = run_bass_kernel_spmd(nc, in_maps, core_ids=list(range(NCORES)))
    out = np.concatenate([res.results[i]["out"] for i in range(NCORES)],
                         axis=0)
    return out.astype(np.float32)


# revision 46
# speedup vs baseline: 1.0310x; 1.0293x over previous
"""Trainium2 Bass kernel for AIMv2FlashAttention2 (packed varlen attention).

Problem: hidden [8192, 1024] = 8 packed sequences x 1024 tokens, dim=1024,
16 heads x 64 head_dim. qkv proj + RoPE (rotate-half) + block-diagonal
softmax attention + out proj.

Strategy: pure data parallelism -- attention is block-diagonal per sequence,
so each of the 8 NeuronCores processes one full sequence locally with
replicated weights. Zero collectives.

v10 structure (exp-floor-paced stream; fused per-step exp):
  - scores for a step live in ONE [128, 2, 1024] PSUM tile (4 banks,
    bufs=1): 8 K=32 QK matmuls on 4 PE row-groups, then a SINGLE
    ScalarE exp over all 2048 columns ((2048+352)/1.2 = 2.0us vs
    2x1.15 split) -- ScalarE stops being the step pacer.
  - per step: QK (2 rounds of 4 row-tiled matmuls) -> fused exp ->
    softmax-sum quad (4 col-tiled M=1 matmuls, concurrent) -> PV
    (2 rounds of 2 col-tiled M=64), PV/sums trail QK by LAG=2 steps.
  - feeder work (qkv chunks, v chunks, per-seg out-proj, norm
    broadcasts) is spliced between attention blocks by a weighted burn
    (2 sites/step); qk feeders yield at quarter-chunk granularity so
    no step absorbs more than ~1us of feeder work (the old full-chunk
    bursts made feeder steps 2x slower than the exp floor).
  - out-proj restructured per segment: as soon as segment (g, ih)
    normalizes, its outT chunk pair (2g, 2g+1) is projected for the 4
    token chunks of q-half ih and accumulated into fp32 y_acc; the
    g==3 contribution adds y_acc and streams the output out per
    (tc, eh) half on alternating HWDGE rings. Only segment (3,1)'s
    projection trails the stream (~7us tail).
  - normalization: sums staged to SBUF (one wide DVE copy, high
    priority), broadcast via one-hot K=128 matmul, reciprocal, single
    multiply evacuates PSUM->outT; drained AFTER the next segment's QK
    block so the Rs matmul never head-of-line blocks the QK/exp pacer.
  - PSUM: scores 1x[128,2048] (4 banks), pv 2x[128,512], sums 1 bank,
    feeder scratch 1 bank.
  - bf16 output (host casts back to fp32): halves the output DMA.
"""

import numpy as np
import ml_dtypes

import concourse.bass as bass
import concourse.bacc as bacc
import concourse.mybir as mybir
import concourse.tile as tile
from concourse.bass import ts

F32 = mybir.dt.float32
F16 = mybir.dt.bfloat16

P = 128
L = 1024          # tokens per sequence / core
DIM = 1024
H = 16            # heads
D = 64            # head dim
NCORES = 8
LAG = 2           # PV trails QK by this many jc steps


def build_nc(dbg=False):
    nc = bacc.Bacc(None)

    xT = nc.declare_dram_parameter("xT", [DIM, L], F16, isOutput=False)
    wqk = nc.declare_dram_parameter("wqk", [16, P, DIM], F16, isOutput=False)
    wv = nc.declare_dram_parameter("wv", [8, P, DIM], F16, isOutput=False)
    wp = nc.declare_dram_parameter("wp", [8, P, DIM], F16, isOutput=False)
    cos4 = nc.declare_dram_parameter("cos4", [P, L], F16, isOutput=False)
    sin4 = nc.declare_dram_parameter("sin4", [P, L], F16, isOutput=False)
    # sel128[k, cpar, m] = 1.0 where k == 32*(2*cpar + m//64): K=128 one-hot
    # matmul replicating softmax-sum rows (at partitions 0/32/64/96) onto
    # the [128, 512] head-pair layout
    sel = nc.declare_dram_parameter("sel", [P, 2, P], F16, isOutput=False)
    # bf16 output: halves the 4MB/core output DMA; host casts to fp32
    out = nc.declare_dram_parameter("out", [L, DIM], F16, isOutput=True)

    Exp = mybir.ActivationFunctionType.Exp
    MUL = mybir.AluOpType.mult
    ADD = mybir.AluOpType.add
    SUB = mybir.AluOpType.subtract

    with tile.TileContext(nc) as tc:
        with (
            tc.tile_pool(name="consts", bufs=1) as consts,
            tc.tile_pool(name="qk", bufs=1) as qkpool,
            tc.tile_pool(name="vmat", bufs=1) as vpool,
            tc.tile_pool(name="outTp", bufs=1) as opool,
            tc.tile_pool(name="small", bufs=1) as small,
            tc.tile_pool(name="xt", bufs=1) as xtp,
            tc.tile_pool(name="wqks", bufs=6) as wqks,
            tc.tile_pool(name="ropetmp", bufs=8) as rtmp,
            tc.tile_pool(name="wmat", bufs=8) as wmat,
            tc.tile_pool(name="wpmat", bufs=8) as wpmat,
            tc.tile_pool(name="probs", bufs=4) as probs,
            tc.tile_pool(name="stag", bufs=2) as stag,
            tc.tile_pool(name="rrec", bufs=4) as rrec,
            tc.tile_pool(name="y", bufs=2) as ypool,
            tc.tile_pool(name="yacc", bufs=1) as yaccp,
            tc.tile_pool(name="psB", bufs=1, space="PSUM") as psB,
            tc.tile_pool(name="psV", bufs=2, space="PSUM") as psV,
            tc.tile_pool(name="psSum", bufs=1, space="PSUM") as psSum,
            tc.tile_pool(name="psF", bufs=1, space="PSUM") as psF,
        ):
            # ---- startup DMAs: xt on sync ring, weights on scalar ring.
            # DMA is ~390 GB/s SHARED across rings; the first matmul (v0)
            # needs only xt0+wv0 and the dc-chain streams behind the xt
            # arrivals, so interleave per-chunk.
            xt_sb = xtp.tile([P, 8, L], F16, tag="xt")
            wv_t = []
            for dc in range(8):
                nc.sync.dma_start(xt_sb[:, dc, :], xT[ts(dc, P), :])
                w = wmat.tile([P, DIM], F16, tag="w", name=f"wv{dc}")
                nc.scalar.dma_start(w[:], wv[dc])
                wv_t.append(w)

            cos_sb = consts.tile([P, L], F16, tag="cos")
            sin_sb = consts.tile([P, L], F16, tag="sin")
            ones_c = consts.tile([P, 1], F16, tag="ones")
            sel_sb = small.tile([P, 2, P], F16, tag="sel")
            nc.scalar.dma_start(cos_sb[:], cos4[:])
            nc.scalar.dma_start(sin_sb[:], sin4[:])
            nc.scalar.dma_start(sel_sb[:], sel[:])
            nc.gpsimd.memset(ones_c[:], 1.0)

            q_sb = qkpool.tile([P, 8, L], F16, tag="q")
            k_sb = qkpool.tile([P, 8, L], F16, tag="k")
            v_sb = vpool.tile([P, 8, H, D], F16, tag="v")
            outT = opool.tile([P, 8, L], F16, tag="o")
            y_acc = yaccp.tile([P, 8, L], F32, tag="ya")

            def v_chunk_gen(tc_):
                """Generator: v for token chunk tc_ via psF halves."""
                for jh in (0, 1):
                    jsl = slice(512 * jh, 512 * jh + 512)
                    V = psF.tile([P, 512], F32, tag="pf", name="Vt")
                    for dc in range(8):
                        nc.tensor.matmul(
                            V[:],
                            lhsT=xt_sb[:, dc, ts(tc_, P)],
                            rhs=wv_t[dc][:, jsl],
                            start=(dc == 0), stop=(dc == 7),
                        )
                        if dc == 3:
                            yield 860 + _UNSAFE
                    with tc.high_priority(offset=30):
                        nc.vector.tensor_copy(
                            v_sb[:, tc_, 8 * jh:8 * jh + 8, :],
                            V[:].rearrange("p (h d) -> p h d", d=D),
                        )
                    yield 900

            # PSUM WAR deps are tile-granular: a tile being evacuated must
            # not receive new writes until the read completes, so ping-pong
            # needs SEPARATE tiles. Pre/tail units cycle their [128,512]
            # halves across four distinct pool slots (psF, psSum, psV x2).
            slot_state = [0]

            def _slot_tile(nm):
                i = slot_state[0] % 4
                slot_state[0] += 1
                if i == 0:
                    return psF.tile([P, 512], F32, tag="pf", name=nm)
                if i == 1:
                    return psSum.tile([P, 512], F32, tag="ps", name=nm)
                return psV.tile([P, 512], F32, tag="pv", name=nm)

            def pre_units(units):
                """Pre-attention chunk units, halves ping-ponged across
                the 4 pool slots: unit i+1's matmuls overlap unit i's
                evacs. c >= 16 -> v chunk c-16, else q/k chunk c.
                Returns epre tiles for q/k units."""
                ev = []
                for c in units:
                    if c >= 16:   # v chunk c-16
                        vc = c - 16
                        for jh in (0, 1):
                            jsl = slice(512 * jh, 512 * jh + 512)
                            V = _slot_tile("Vpre")
                            for dc in range(8):
                                nc.tensor.matmul(
                                    V[:],
                                    lhsT=xt_sb[:, dc, ts(vc, P)],
                                    rhs=wv_t[dc][:, jsl],
                                    start=(dc == 0), stop=(dc == 7),
                                )
                            nc.scalar.copy(
                                v_sb[:, vc, 8 * jh:8 * jh + 8, :],
                                V[:].rearrange("p (h d) -> p h d", d=D),
                            )
                    else:         # q/k chunk c
                        wt = wqk_pre.pop(c)
                        e = rtmp.tile([P, L], F16, tag="rt", name="epre")
                        for th in (0, 1):
                            tsl = slice(512 * th, 512 * th + 512)
                            S = _slot_tile("Spre")
                            for dc in range(8):
                                nc.tensor.matmul(
                                    S[:],
                                    lhsT=wt[:, ts(dc, P)],
                                    rhs=xt_sb[:, dc, tsl],
                                    start=(dc == 0), stop=(dc == 7),
                                )
                            nc.scalar.copy(e[:, tsl], S[:])
                        ev.append(e)
                return ev

            def v_rest(vcs):
                for tc_ in vcs:
                    yield from v_chunk_gen(tc_)

            def rope_pair(c, U, Lp):
                tgt = q_sb if c < 8 else k_sb
                ci = c if c < 8 else c - 8
                t1 = rtmp.tile([P, L], F16, tag="rt", name="t1")
                t2 = rtmp.tile([P, L], F16, tag="rt", name="t2")
                nc.vector.tensor_tensor(tgt[:, ci, :], U[:], cos_sb[:], MUL)
                nc.vector.tensor_tensor(t1[:], Lp[:], sin_sb[:], MUL)
                nc.vector.tensor_tensor(
                    tgt[:, ci, :], tgt[:, ci, :], t1[:], SUB)
                yield 250
                nc.vector.tensor_tensor(
                    tgt[:, ci + 1, :], Lp[:], cos_sb[:], MUL)
                nc.vector.tensor_tensor(t2[:], U[:], sin_sb[:], MUL)
                nc.vector.tensor_tensor(
                    tgt[:, ci + 1, :], tgt[:, ci + 1, :], t2[:], ADD)
                yield 250

            def qk_chunk_pair(c):
                """Generator producing q or k chunks (c, c+1) via psF
                halves, quarter-chunk burn granularity."""
                ev = []
                for cc in (c, c + 1):
                    wt = wqks.tile([P, DIM], F16, tag="wqk")
                    nc.sync.dma_start(wt[:], wqk[cc])
                    e = rtmp.tile([P, L], F16, tag="rt")
                    for th in (0, 1):
                        tsl = slice(512 * th, 512 * th + 512)
                        S = psF.tile([P, 512], F32, tag="pf", name="Sqk")
                        # mid-chain yields are UNSAFE (+_UNSAFE): the psF
                        # tile is half-written, so no other psF user may
                        # be issued until the chain resumes -- burn_one
                        # tracks this and to_safe() runs before the norm's
                        # Rs is issued.
                        for dc in range(8):
                            nc.tensor.matmul(
                                S[:],
                                lhsT=wt[:, ts(dc, P)],
                                rhs=xt_sb[:, dc, tsl],
                                start=(dc == 0), stop=(dc == 7),
                            )
                            if dc == 3:
                                yield 860 + _UNSAFE
                        with tc.high_priority(offset=30):
                            nc.vector.tensor_copy(e[:, tsl], S[:])
                        yield 900
                    ev.append(e)
                yield from rope_pair(c, ev[0], ev[1])



            def drain(feeder):
                if feeder is not None:
                    for _ in feeder:
                        pass

            def norm_evac(g, ih, pvAB, pvCD, st):
                """Generator: normalize+evacuate segment (g, ih) given its
                staged sums tile st. Spliced into the NEXT segment."""
                isl = slice(512 * ih, 512 * ih + 512)
                rs = []
                for cc in (2 * g, 2 * g + 1):
                    Rs = psF.tile([P, 512], F32, tag="pf", name="Rs")
                    nc.tensor.matmul(
                        Rs[:], lhsT=sel_sb[:, cc % 2, :], rhs=st[:],
                        start=True, stop=True,
                    )
                    Rr = rrec.tile([P, 512], F32, tag="rr")
                    with tc.high_priority(offset=30):
                        nc.vector.reciprocal_approx_fast(out=Rr[:],
                                                         in_=Rs[:])
                    rs.append(Rr)
                yield 440
                for cc, Rr, pvt in ((2 * g, rs[0], pvAB),
                                    (2 * g + 1, rs[1], pvCD)):
                    with tc.high_priority(offset=30):
                        nc.vector.tensor_tensor(
                            outT[:, cc, isl], pvt[:], Rr[:], MUL)
                yield 0

            wp_t = {}

            def proj_quad(ph, ih):
                """Generator: out-proj of outT chunk quad ccs=4ph..4ph+3
                for q-half ih's 4 token chunks. Two phases per (tc, eh):
                ph0 initializes fp32 y_acc, ph1 adds it and streams the
                output half out on alternating rings. One DVE op per unit
                (vs one per chunk-pair) keeps the psF ring chain short."""
                ccs = range(4 * ph, 4 * ph + 4)
                if ih == 0:
                    for cc in ccs:
                        w = wpmat.tile([P, DIM], F16, tag="wp",
                                       name=f"wp{cc}")
                        nc.sync.dma_start(w[:], wp[cc])
                        wp_t[cc] = w
                    yield 0
                tail = (ph == 1 and ih == 1)
                for tc_ in range(4 * ih, 4 * ih + 4):
                    ysb = (ypool.tile([P, DIM], F16, tag="y2", name="ysb")
                           if ph == 1 else None)
                    for eh in (0, 1):
                        esl = slice(512 * eh, 512 * eh + 512)
                        if tail:
                            Y = _slot_tile("Ytl")[:]
                        else:
                            Y = psF.tile([P, 512], F32, tag="pf",
                                         name="Yp")[:]
                        for ix, cc in enumerate(ccs):
                            nc.tensor.matmul(
                                Y,
                                lhsT=outT[:, cc, ts(tc_, P)],
                                rhs=wp_t[cc][:, esl],
                                start=(ix == 0), stop=(ix == 3),
                            )
                        with tc.high_priority(offset=30):
                            if ph == 0:
                                nc.vector.tensor_copy(
                                    y_acc[:, tc_, esl], Y)
                            else:
                                nc.vector.tensor_tensor(
                                    ysb[:, esl], Y, y_acc[:, tc_, esl],
                                    ADD)
                        if ph == 1:
                            (nc.sync if eh == 0 else nc.scalar).dma_start(
                                out[ts(tc_, P), esl], ysb[:, esl])
                        yield 1100

            def attention_stream(schedule):
                """Continuous stream over 8 segments (quad g, query-half
                ih). Per step: QK rounds + ONE fused exp; sums quad + PV
                trail by LAG. `schedule` is a list of (t_start, generator)
                feeder gates (must be sorted by t_start); feeder work is
                burned around the attention blocks to fill PE slack under
                the exp-latency step floor."""
                active = []
                pending = [None]
                unsafe = [False]   # active[0] suspended mid-psF-tile?

                def burn_one():
                    while True:
                        if not active:
                            return
                        w = next(active[0], _SENT)
                        if w is _SENT:
                            active.pop(0)
                            unsafe[0] = False
                        else:
                            unsafe[0] = w >= _UNSAFE
                            if (w % _UNSAFE) >= 300:
                                return

                def to_safe():
                    # resume active[0] past any half-written psF tile
                    # before another psF user (the norm's Rs) is issued
                    while unsafe[0] and active:
                        w = next(active[0], _SENT)
                        if w is _SENT:
                            active.pop(0)
                            unsafe[0] = False
                        else:
                            unsafe[0] = w >= _UNSAFE

                def drain_pending():
                    to_safe()
                    while pending[0] is not None:
                        if next(pending[0], _SENT) is _SENT:
                            pending[0] = None

                nseg = 8
                prbs = {}
                cur = {}
                for t in range(8 * nseg + LAG):
                    while schedule and schedule[0][0] <= t:
                        active.append(schedule.pop(0)[1])
                    s_pv, pj = divmod(t - LAG, 8)
                    # drain the previous segment's norm BEFORE this step's
                    # QK: with the high-priority st copy + to_safe, the Rs
                    # matmuls land in the exp(t-1) window instead of
                    # splitting the next score block
                    if 0 <= s_pv < nseg and pj == 0:
                        drain_pending()
                        cur['pvAB'] = psV.tile([P, 512], F32, tag="pv",
                                               name="pvAB")
                        cur['pvCD'] = psV.tile([P, 512], F32, tag="pv",
                                               name="pvCD")
                        cur['sum4'] = psSum.tile([P, 512], F32, tag="ps",
                                                 name="sum4")
                    burn_one()
                    s_qk, jc = divmod(t, 8)
                    if s_qk < nseg:
                        g, ih = divmod(s_qk, 2)
                        isl = slice(512 * ih, 512 * ih + 512)
                        S2 = psB.tile([P, 2, L], F32, tag="pb", name="S2")
                        s_of = {0: (0, 0), 1: (0, 512),
                                2: (1, 0), 3: (1, 512)}
                        for lo in (0, 1):   # up halves then lo halves
                            for j in range(4):
                                hf, co = s_of[j]
                                psl = slice(32 * j, 32 * j + 32)
                                nc.tensor.matmul(
                                    S2[:, hf, co:co + 512],
                                    lhsT=k_sb[psl, 2 * g + lo, ts(jc, P)],
                                    rhs=q_sb[psl, 2 * g + lo, isl],
                                    start=(lo == 0), stop=(lo == 1),
                                    tile_position=(32 * j, 0),
                                )
                        prb = probs.tile([P, 2, L], F16, tag="pr")
                        nc.scalar.activation(prb[:], S2[:], Exp,
                                             scale=0.125)
                        prbs[t] = prb
                    burn_one()
                    if 0 <= s_pv < nseg:
                        g, ih = divmod(s_pv, 2)
                        heads = [4 * g + j for j in range(4)]
                        prb = prbs.pop(t - LAG)
                        p_of = {0: (0, 0), 1: (0, 512),
                                2: (1, 0), 3: (1, 512)}
                        # softmax-sum quad first: 4 col-tiled M=1 matmuls
                        # issued back-to-back run concurrently
                        for j in range(4):
                            hf, co = p_of[j]
                            nc.tensor.matmul(
                                cur['sum4'][32 * j:32 * j + 1, :],
                                lhsT=ones_c[:],
                                rhs=prb[:, hf, co:co + 512],
                                start=(pj == 0), stop=(pj == 7),
                                tile_position=(0, 32 * j),
                                skip_group_check=True,
                            )
                        for j in range(4):
                            hf, co = p_of[j]
                            pvt = cur['pvAB'] if j < 2 else cur['pvCD']
                            ro = (j % 2) * D
                            nc.tensor.matmul(
                                pvt[ro:ro + D, :],
                                lhsT=v_sb[:, pj, heads[j], :],
                                rhs=prb[:, hf, co:co + 512],
                                start=(pj == 0), stop=(pj == 7),
                                tile_position=(0, ro),
                                skip_group_check=True,
                            )
                        if pj == 7:
                            st = stag.tile([P, 512], F16, tag="st",
                                           name="st")
                            # high priority: jump the DVE queue so the
                            # next boundary's Rs never waits on this copy
                            with tc.high_priority(offset=40):
                                nc.vector.tensor_copy(st[:], cur['sum4'][:])
                            pending[0] = norm_evac(
                                g, ih, cur['pvAB'], cur['pvCD'], st)
                # retire the last segment's normalization, then leftovers.
                # Gates > loop-end (e.g. the final proj) are appended only
                # AFTER the drain: a reader of outT issued before the norm
                # mult would see stale data (WAR -- dep tracking orders by
                # issue order).
                drain_pending()
                while schedule:
                    active.append(schedule.pop(0)[1])
                while active:
                    burn_one()

            # ---------------- pipeline ----------------
            # pre-attention: v chunks 0-5 + q/k chunks 0,1 (q) and 8,9 (k).
            # The pre window is DMA-paced early, so the extra v chunks ride
            # for ~free and the in-stream feeder load drops to what fits
            # under the 2.8us exp-floor slack. wqk chunks prefetched so no
            # unit waits on a just-in-time weight DMA.
            wqk_pre = {}
            for c in (0, 1, 8, 9):
                wt0 = wqks.tile([P, DIM], F16, tag="wqk", name=f"wqkp{c}")
                nc.sync.dma_start(wt0[:], wqk[c])
                wqk_pre[c] = wt0
            ev_q = pre_units([16, 17, 0, 1])          # v0, v1, q0, q1
            rq = rope_pair(0, ev_q[0], ev_q[1])
            next(rq, None)
            ev_k = pre_units([18, 8, 9])              # v2, k0, k1
            drain(rq)
            rk = rope_pair(8, ev_k[0], ev_k[1])
            next(rk, None)
            pre_units([19, 20, 21])                   # v3, v4, v5
            drain(rk)

            # qk pair c is needed by segment (c % 8) // 2 * 2 (+1), i.e.
            # q/k quad g by t = 16g; gates spread each ~7us pair so the
            # per-step feeder load stays under the exp-floor slack.
            schedule = [
                (0, v_rest(range(6, 8))),
                (0, qk_chunk_pair(2)),        # q chunks 2,3 (need t=16)
                (5, qk_chunk_pair(10)),       # k chunks 2,3 (need t=16)
                (18, qk_chunk_pair(4)),       # q chunks 4,5 (need t=32)
                (25, qk_chunk_pair(12)),      # k chunks 4,5 (need t=32)
                (27, proj_quad(0, 0)),        # ccs 0-3, tcs 0-3
                (35, proj_quad(0, 1)),        # ccs 0-3, tcs 4-7
                (36, qk_chunk_pair(6)),       # q chunks 6,7 (need t=48)
                (42, qk_chunk_pair(14)),      # k chunks 6,7 (need t=48)
                (59, proj_quad(1, 0)),        # ccs 4-7, tcs 0-3
                (99, proj_quad(1, 1)),   # tail: after the final norm drain
            ]
            attention_stream(schedule)

    nc.compile()
    return nc


_SENT = object()
_UNSAFE = 10000


def _qk_perm():
    """Column permutation for q (or k) weights: chunk 2g = upper halves
    (d 0:32) of heads 4g..4g+3, chunk 2g+1 = lower halves."""
    perm = []
    for g in range(4):
        for d0 in (0, 32):
            for j in range(4):
                h = 4 * g + j
                perm.extend(h * D + d for d in range(d0, d0 + 32))
    return np.asarray(perm)


def prep_shards(hidden_states, cos, sin, w_qkv, b_qkv, w_proj, b_proj,
                cu_seqlens=None):
    """Build the per-core input maps (host-side, numpy)."""
    perm = _qk_perm()
    wq = w_qkv[:, :DIM][:, perm]
    wk = w_qkv[:, DIM:2 * DIM][:, perm]
    wqk_cols = np.concatenate([wq, wk], axis=1)            # [1024, 2048]
    # Wqk[c, dp, dc*128 + j] = wqk_cols[dc*128 + dp, c*128 + j]
    Wqk = np.ascontiguousarray(
        wqk_cols.reshape(8, P, 16, P).transpose(2, 1, 0, 3).reshape(16, P, DIM)
    ).astype(ml_dtypes.bfloat16)
    Wv = np.ascontiguousarray(
        w_qkv[:, 2 * DIM:].reshape(8, P, DIM)).astype(ml_dtypes.bfloat16)
    Wp = np.ascontiguousarray(
        w_proj.reshape(8, P, DIM)).astype(ml_dtypes.bfloat16)

    in_maps = []
    for i in range(NCORES):
        sl = slice(i * L, (i + 1) * L)
        xT = np.ascontiguousarray(
            hidden_states[sl].T).astype(ml_dtypes.bfloat16)
        cosT = cos[sl, :D // 2].T.astype(np.float32)       # [32, 1024]
        sinT = sin[sl, :D // 2].T.astype(np.float32)
        cos4 = np.ascontiguousarray(
            np.tile(cosT, (4, 1))).astype(ml_dtypes.bfloat16)
        sin4 = np.ascontiguousarray(
            np.tile(sinT, (4, 1))).astype(ml_dtypes.bfloat16)
        in_maps.append({
            "xT": xT, "wqk": Wqk, "wv": Wv, "wp": Wp,
            "cos4": cos4, "sin4": sin4, "sel": _sel_mat(),
        })
    return in_maps


def _sel_mat():
    sel = np.zeros((P, 2, P), ml_dtypes.bfloat16)
    for cpar in range(2):
        for m in range(P):
            sel[32 * (2 * cpar + m // D), cpar, m] = 1.0
    return sel


_NC_CACHE = {}


def kernel(hidden_states, cos, sin, w_qkv, b_qkv, w_proj, b_proj,
           cu_seqlens=None, **_unused):
    hidden_states = np.asarray(hidden_states)
    assert hidden_states.shape == (NCORES * L, DIM)

    from concourse.bass_utils import run_bass_kernel_spmd

    if "nc" not in _NC_CACHE:
        _NC_CACHE["nc"] = build_nc()
    nc = _NC_CACHE["nc"]

    in_maps = prep_shards(np.asarray(hidden_states), np.asarray(cos),
                          np.asarray(sin), np.asarray(w_qkv),
                          np.asarray(b_qkv), np.asarray(w_proj),
                          np.asarray(b_proj))
    res = run_bass_kernel_spmd(nc, in_maps, core_ids=list(range(NCORES)))
    out = np.concatenate([res.results[i]["out"] for i in range(NCORES)],
                         axis=0)
    return out.astype(np.float32)
